# revision 49
# baseline (speedup 1.0000x reference)
"""Multi-head attention (B=4, S=2048, D=2048, H=16) on 8 trn2 NeuronCores.

Sharding: 4 head-groups x 2 batch-groups. Core c handles heads
[(c//2)*4, (c//2)*4+4) for batches [(c%2)*2, (c%2)*2+2). Each core computes
its heads' Q/K/V projections, full causal+padding-masked attention, and a
partial output projection; the host sums the 4 partial outputs per batch.

All matmuls are bf16. Attention uses a transposed-scores layout st[k, q] so
the key-padding mask folds into the exp() bias (per-partition) and exp
tiles feed the attn@V matmul directly as the moving operand. Causal
masking: diagonal 128x512 blocks only compute the causally-live q-range
(scores, attn@V and denominator matmuls all shrink) and the one partially-
masked [128,128] triangle is zeroed by a DVE multiply with a 0/1 triangle
tile. Softmax denominators come from an all-ones stationary matmul; rows
whose causally-visible keys are all masked get the reference's uniform-
attention fallback via a host-precomputed indicator + mean-of-V fixup.

The attention phase alone over-commits the ACT (exp) and DVE (drains,
reciprocal) engines relative to the PE, so phases of the TWO batches are
software-pipelined at emission level: attn(b0) blocks interleave with
Q/K/V-projection chunks of b1, and O-proj(b0) chunks interleave with
attn(b1). The PE then always has queued matmuls while ACT/DVE work off
their backlog. qt/kt/v/ot live double-buffered per batch; projections run
in 2-head passes so Q/K + V + O share 4 PSUM banks, attention the other 4.

Inputs are host-retiled into contiguous [128, 512]-class blocks so every
DMA is a few large descriptors. Output partials are written bf16 and
summed on host in f32.
"""

import os
import sys

import numpy as np

sys.path.insert(0, "/opt/trn_rl_repo")

B, S, D, H, DK = 4, 2048, 2048, 16, 128
NHG = 4  # head groups (cores along head axis)
NBG = 2  # batch groups
HPC = H // NHG  # heads per core = 4
BPC = B // NBG  # batches per core = 2
NI = D // 128  # contraction blocks = 16
NSC = S // 512  # 512-wide s-chunks = 4
NST = S // 128  # 128-wide s-tiles = 16
SCALE = 1.0 / float(np.sqrt(DK))
NEGB = -30000.0

_cache = {}


def _register_ntff_hook():
    """The agent image lacks antenv.axon_hooks; register the NTFF profile
    hook manually so trace=True can report HW exec time."""
    import types

    if "antenv.axon_hooks" in sys.modules:
        return
    try:
        import trn_agent_boot.trn_boot as _tb

        hook = _tb._ntff_profile_via_ctypes("/opt/axon/libaxon_pjrt.so")
    except Exception:
        hook = None
    m = types.ModuleType("antenv.axon_hooks")
    m.get_axon_ntff_profile_hook = lambda: hook
    m.set_axon_ntff_profile_hook = lambda h: None
    sys.modules["antenv.axon_hooks"] = m


def _split_waits(nc):
    """This container's walrus accepts a single sync-wait per instruction.
    Hoist extra waits onto EventSemaphore instructions placed immediately
    before the over-subscribed instruction on the same engine."""
    import concourse.mybir as mb

    ctr = 0
    for f in nc.m.functions:
        for blk in f.blocks:
            new = []
            for inst in blk.instructions:
                si = inst.sync_info
                waits = list(si.on_wait) if (si and si.on_wait) else []
                if len(waits) > 1:
                    for w in waits[:-1]:
                        ctr += 1
                        ev = mb.InstEventSemaphore(
                            name=f"WSPLIT-{ctr}", ins=[], outs=[]
                        )
                        ev.engine = inst.engine
                        ev.sync_info = mb.SyncInfo(on_wait=[w], on_update=[])
                        new.append(ev)
                    si.on_wait = [waits[-1]]
                new.append(inst)
            blk.instructions[:] = new
    return ctr


def _merge_streams(attn, partner, prereqs=None):
    """attn/partner: lists of (pe_ns, emit_fn). Emit in an order that keeps
    cumulative PE-time balanced, preserving per-stream order. prereqs[j]
    (if given) is the number of partner chunks that MUST be emitted before
    attn chunk j: the tile framework only syncs reads against writes that
    are already in the program, so a consumer emitted before its producer
    reads garbage silently."""
    ia = ip = 0
    ca = cp = 0.0
    while ia < len(attn) or ip < len(partner):
        need = prereqs[ia] if (prereqs and ia < len(attn)) else 0
        can_a = ia < len(attn) and ip >= need
        can_p = ip < len(partner)
        if can_a and (not can_p or ca <= cp):
            ns, fn = attn[ia]
            fn()
            ca += ns
            ia += 1
        else:
            ns, fn = partner[ip]
            fn()
            cp += ns
            ip += 1


def _build_program():
    import concourse.bass as bass
    import concourse.mybir as mybir
    import concourse.tile as tile
    from contextlib import ExitStack

    f32 = mybir.dt.float32
    bf16 = mybir.dt.bfloat16
    EXP = mybir.ActivationFunctionType.Exp
    LN = mybir.ActivationFunctionType.Ln
    ADD = mybir.AluOpType.add
    MUL = mybir.AluOpType.mult

    nc = bass.Bass()
    # host-retiled contiguous blocks
    xbd = nc.dram_tensor(
        "xtb", [BPC, NI, NSC, 128, 512], bf16, kind="ExternalInput"
    ).ap()
    # per 2-head pass: wq cols (256) then wk cols (256) per i-block
    wqkd = nc.dram_tensor(
        "wqkt", [2, NI, 128, 512], bf16, kind="ExternalInput"
    ).ap()
    wvd = nc.dram_tensor(
        "wvt", [NI, 128, HPC * DK], bf16, kind="ExternalInput"
    ).ap()
    wod = nc.dram_tensor(
        "wott", [NSC, HPC, 128, 512], bf16, kind="ExternalInput"
    ).ap()
    mbd = nc.dram_tensor("mb", [BPC, 128, NST], f32, kind="ExternalInput").ap()
    zmd = nc.dram_tensor("zmh", [BPC, 128, S], bf16, kind="ExternalInput").ap()
    trid = nc.dram_tensor("tri", [128, 128], bf16, kind="ExternalInput").ap()
    mvd = nc.dram_tensor("meanv", [BPC, 128, HPC], f32, kind="ExternalInput").ap()
    onesd = nc.dram_tensor("ones", [128, 128], bf16, kind="ExternalInput").ap()
    outd = nc.dram_tensor("out", [BPC, S, D], bf16, kind="ExternalOutput").ap()

    with tile.TileContext(nc) as tc, ExitStack() as ctx:
        singles = ctx.enter_context(tc.tile_pool(name="singles", bufs=1))
        pers = ctx.enter_context(tc.tile_pool(name="pers", bufs=1))
        xs = ctx.enter_context(tc.tile_pool(name="xs", bufs=4))
        # one [128,512] tile per i-block so a matmul only waits on its own
        # 128KB weight DMA, not the whole pass's 2MB
        wqp = ctx.enter_context(tc.tile_pool(name="wqp", bufs=NI))
        expp = ctx.enter_context(tc.tile_pool(name="expp", bufs=4))
        pdp = ctx.enter_context(tc.tile_pool(name="pdp", bufs=2))
        outp = ctx.enter_context(tc.tile_pool(name="outp", bufs=4))
        wop = ctx.enter_context(tc.tile_pool(name="wop", bufs=2))
        # PSUM: 4 banks for all projections, 4 for attention
        psP = ctx.enter_context(tc.tile_pool(name="psP", bufs=1, space="PSUM"))
        psS = ctx.enter_context(tc.tile_pool(name="psS", bufs=2, space="PSUM"))
        psO = ctx.enter_context(tc.tile_pool(name="psO", bufs=1, space="PSUM"))
        psD = ctx.enter_context(tc.tile_pool(name="psD", bufs=1, space="PSUM"))

        # constants load on the idle gpsimd queue so they never gate the
        # first matmuls' x/w DMAs on the sync queue
        tri_sb = singles.tile([128, 128], bf16)
        nc.gpsimd.dma_start(out=tri_sb, in_=trid)
        mb_sb = singles.tile([128, BPC, NST], f32)
        mv_sb = singles.tile([128, BPC, HPC], f32)
        zm_sb = singles.tile([128, BPC, S], bf16)
        for b in range(BPC):
            nc.gpsimd.dma_start(out=mb_sb[:, b, :], in_=mbd[b])
            nc.gpsimd.dma_start(out=mv_sb[:, b, :], in_=mvd[b])
            nc.gpsimd.dma_start(out=zm_sb[:, b, :], in_=zmd[b])
        ones_sb = singles.tile([128, 128], bf16)
        nc.gpsimd.dma_start(out=ones_sb, in_=onesd)
        # V weights live in SBUF for the whole kernel (same for both
        # batches); loaded from the schedule after the first QK chunks so
        # the startup DMA burst doesn't stall the first matmuls
        wv_sb = singles.tile([128, NI, HPC * DK], bf16)

        def load_wv():
            for ib in range(NI):
                nc.gpsimd.dma_start(out=wv_sb[:, ib, :], in_=wvd[ib])

        # double-buffered per-batch activations
        qt = [[pers.tile([128, S], bf16, name=f"qt{bb}_{h}") for h in range(HPC)]
              for bb in range(BPC)]
        kt = [[pers.tile([128, S], bf16, name=f"kt{bb}_{h}") for h in range(HPC)]
              for bb in range(BPC)]
        ot = [[pers.tile([128, S], bf16, name=f"ot{bb}_{h}") for h in range(HPC)]
              for bb in range(BPC)]
        v_sb = [pers.tile([128, NST, HPC * DK], bf16, name=f"v{bb}")
                for bb in range(BPC)]

        MM = 0.4167 * 512  # ns per full 512-row matmul

        # ---- Q/K projection chunks: 2-head passes, 4 PSUM banks ----
        def qk_stream(b):
            state = {}

            def start_pass_sc(pas, sc):
                if sc == 0:
                    state["w"] = [
                        wqp.tile([128, 512], bf16, name="wq_blk")
                        for _ in range(NI)
                    ]
                state["pq"] = [psP.tile([128, 512], f32, name=f"pp{j}")
                               for j in range(2)]
                state["pk"] = [psP.tile([128, 512], f32, name=f"pp{j + 2}")
                               for j in range(2)]

            def emit_ibq(pas, sc, ibq):
                if ibq == 0:
                    start_pass_sc(pas, sc)
                w, pq, pk = state["w"], state["pq"], state["pk"]
                for ib in range(ibq * 4, ibq * 4 + 4):
                    if sc == 0:
                        # smear the weight loads (second queue) so the
                        # first matmuls aren't gated on a 2MB burst
                        nc.scalar.dma_start(out=w[ib], in_=wqkd[pas, ib])
                    xt_blk = xs.tile([128, 512], bf16, name="xb_blk")
                    # S1 (b0 pass0) runs without an attention partner and
                    # is DMA-paced: spread x-loads over both hwdge queues.
                    # NEVER route x to the ACT queue outside S1 - it is
                    # saturated with exp there and dispatches cost 640ns.
                    xq = (
                        nc.scalar
                        if (b == 0 and pas == 0 and ib % 2 == 0)
                        else nc.sync
                    )
                    xq.dma_start(out=xt_blk, in_=xbd[b, ib, sc])
                    wqk_blk = w[ib]
                    for m in range(2):
                        nc.tensor.matmul(
                            pq[m][:, :],
                            wqk_blk[:, m * DK : (m + 1) * DK],
                            xt_blk[:, :],
                            start=(ib == 0),
                            stop=(ib == NI - 1),
                        )
                        nc.tensor.matmul(
                            pk[m][:, :],
                            wqk_blk[:, 256 + m * DK : 256 + (m + 1) * DK],
                            xt_blk[:, :],
                            start=(ib == 0),
                            stop=(ib == NI - 1),
                        )
                if ibq == 3:
                    ssl = slice(sc * 512, (sc + 1) * 512)
                    for m in range(2):
                        h = pas * 2 + m
                        nc.scalar.copy(qt[b][h][:, ssl], pq[m][:, :])
                        nc.vector.tensor_copy(kt[b][h][:, ssl], pk[m][:, :])

            return [
                (16 * MM, (lambda pas=pas, sc=sc, ibq=ibq:
                           emit_ibq(pas, sc, ibq)))
                for pas in range(2) for sc in range(NSC) for ibq in range(4)
            ]

        # ---- V projection chunks: one 512-token s-chunk per pass ----
        def v_stream(b):
            state = {}

            def emit_ibq(pas, ibq):
                if ibq == 0:
                    state["pv"] = [psP.tile([128, 512], f32, name=f"pp{j}")
                                   for j in range(4)]
                pv = state["pv"]
                for ib in range(ibq * 4, ibq * 4 + 4):
                    xv_blk = xs.tile([128, 512], bf16, name="xb_blk")
                    xq = nc.scalar if (b == 0 and ib % 2 == 0) else nc.sync
                    xq.dma_start(out=xv_blk, in_=xbd[b, ib, pas])
                    for j in range(4):
                        nc.tensor.matmul(
                            pv[j][:, :],
                            xv_blk[:, j * 128 : (j + 1) * 128],
                            wv_sb[:, ib, :],
                            start=(ib == 0),
                            stop=(ib == NI - 1),
                        )
                if ibq == 3:
                    for j in range(4):
                        nc.scalar.copy(v_sb[b][:, pas * 4 + j, :], pv[j][:, :])

            return [
                (16 * MM, (lambda pas=pas, ibq=ibq: emit_ibq(pas, ibq)))
                for pas in range(NSC) for ibq in range(4)
            ]

        # ---- attention groups: one (h, qc) chunk each ----
        def attn_stream(b):
            state = {}

            def norm_part(h, sl):
                # normalize an ot[b][h] slice once its pd chunks landed;
                # pd_sb already includes +1 on all-masked rows (host
                # indicator). 1/pd = exp(-ln(pd)) on the ACT tables: DVE
                # InstReciprocal costs 6.5us per [128,1024] and clogs the
                # vector FIFO
                pd_sb = state[("pd", h)]
                nc.scalar.activation(
                    out=pd_sb[:, sl], in_=pd_sb[:, sl], func=LN
                )
                nc.scalar.activation(
                    out=pd_sb[:, sl], in_=pd_sb[:, sl], func=EXP, scale=-1.0
                )
                nc.vector.tensor_tensor(
                    ot[b][h][:, sl], ot[b][h][:, sl], pd_sb[:, sl], MUL
                )
                nc.vector.scalar_tensor_tensor(
                    out=ot[b][h][:, sl],
                    in0=zm_sb[:, b, sl],
                    scalar=mv_sb[:, b, h : h + 1],
                    in1=ot[b][h][:, sl],
                    op0=MUL,
                    op1=ADD,
                )

            def emit_group(h, qc):
                if qc == 0:
                    state[("pd", h)] = pdp.tile([128, S], f32, name="pd_sb")
                pd_sb = state[("pd", h)]
                nkb = (qc + 1) * 4
                po = psO.tile([128, 512], f32, name="po")
                pd = psD.tile([128, 512], f32, name="pd")
                for kb in range(nkb):
                    ps = psS.tile([128, 512], f32, name="ps")
                    p = kb - qc * 4
                    lo = max(p, 0) * 128  # causally-live q-range start
                    nc.tensor.matmul(
                        ps[:, lo:],
                        kt[b][h][:, kb * 128 : (kb + 1) * 128],
                        qt[b][h][:, qc * 512 + lo : (qc + 1) * 512],
                        start=True,
                        stop=True,
                    )
                    e = expp.tile([128, 512], bf16, name="e")
                    nc.scalar.activation(
                        out=e[:, lo:],
                        in_=ps[:, lo:],
                        func=EXP,
                        bias=mb_sb[:, b, kb : kb + 1],
                        scale=SCALE,
                    )
                    if p >= 0:
                        # zero the future-key triangle of the partially-
                        # masked [128,128] sub-block
                        nc.vector.tensor_tensor(
                            e[:, lo : lo + 128],
                            e[:, lo : lo + 128],
                            tri_sb[:, :],
                            MUL,
                        )
                    nc.tensor.matmul(
                        po[:, lo:],
                        v_sb[b][:, kb, h * DK : (h + 1) * DK],
                        e[:, lo:],
                        start=(kb == 0),
                        stop=(kb == nkb - 1),
                    )
                    nc.tensor.matmul(
                        pd[:, lo:],
                        ones_sb[:, :],
                        e[:, lo:],
                        start=(kb == 0),
                        stop=(kb == nkb - 1),
                    )
                qsl = slice(qc * 512, (qc + 1) * 512)
                nc.vector.tensor_copy(ot[b][h][:, qsl], po[:, :])
                nc.vector.tensor_tensor(
                    pd_sb[:, qsl], pd[:, :], zm_sb[:, b, qsl], ADD
                )
                # spread the normalize: everything but the last 512-col
                # chunk is done before qc3, so the tail latency between the
                # last attn block and its O-projection consumer is short
                if qc == 1:
                    norm_part(h, slice(0, 1024))
                elif qc == 2:
                    norm_part(h, slice(1024, 1536))
                elif qc == 3:
                    norm_part(h, slice(1536, 2048))

            ROWS = 0.4167 * 128
            return [
                ((3 * (qc + 1) * 4 * 4 - 3 * (1 + 2 + 3)) * ROWS,
                 (lambda h=h, qc=qc: emit_group(h, qc)))
                for h in range(HPC) for qc in range(NSC)
            ]

        # ---- output projection chunks ----
        def o_stream(b):
            state = {}

            def emit_stq(ec, stq):
                if stq == 0:
                    w = wop.tile([128, HPC, 512], bf16, name="wot_blk")
                    state["w"] = w
                    for hd in range(HPC):
                        nc.sync.dma_start(out=w[:, hd, :], in_=wod[ec, hd])
                w = state["w"]
                for st in range(stq * 4, stq * 4 + 4):
                    pf = psP.tile([128, 512], f32, name=f"pp{st % 4}")
                    for h in range(HPC):
                        nc.tensor.matmul(
                            pf[:, :],
                            ot[b][h][:, st * 128 : (st + 1) * 128],
                            w[:, h, :],
                            start=(h == 0),
                            stop=(h == HPC - 1),
                        )
                    ob = outp.tile([128, 512], bf16, name="ob")
                    if (st + ec) % 2 == 0:
                        nc.vector.tensor_copy(ob[:, :], pf[:, :])
                    else:
                        nc.scalar.copy(ob[:, :], pf[:, :])
                    # b1's O-proj runs solo at the end with the ACT queue
                    # idle: split its output flush across both hwdge queues
                    oq = nc.scalar if (b == 1 and st % 2 == 0) else nc.sync
                    oq.dma_start(
                        out=outd[
                            b,
                            st * 128 : (st + 1) * 128,
                            ec * 512 : (ec + 1) * 512,
                        ],
                        in_=ob[:, :],
                    )

            return [
                (16 * MM, (lambda ec=ec, stq=stq: emit_stq(ec, stq)))
                for ec in range(NSC) for stq in range(4)
            ]

        # ---- schedule ----
        # S1: head-pair 0 Q/K + V projections of b0 (PE-only warmup)
        qk0 = qk_stream(0)
        for _, fn in qk0[:2]:
            fn()
        load_wv()
        for _, fn in qk0[2:16]:
            fn()
        for _, fn in v_stream(0):
            fn()
        # S2: attention b0 (heads 0,1 first) interleaved with the rest of
        # b0's Q/K projection and b1's Q/K projection. Heads 2,3 need the
        # partner's first 16 chunks (b0 head-pair-1 Q/K) emitted first.
        attn0 = attn_stream(0)
        pre2 = [0 if j < 8 else 4 * (j % NSC) + 4 for j in range(len(attn0))]
        _merge_streams(attn0, qk0[16:] + qk_stream(1), pre2)
        # S3: attention b1 interleaved with b1's V projection and b0's
        # O-projection (balances partner PE-time across S2/S3). Group
        # (h, qc) reads v_sb[b1] s-tiles 0..4qc+3, written by the first
        # 4(qc+1) partner chunks.
        attn1 = attn_stream(1)
        pre3 = [4 * (j % NSC) + 4 for j in range(len(attn1))]
        _merge_streams(attn1, v_stream(1) + o_stream(0), pre3)
        # S4: O-projection b1
        for _, fn in o_stream(1):
            fn()

    _split_waits(nc)
    return nc


def _host_prep(x, attention_mask, w_q, w_k, w_v, w_o):
    x = np.asarray(x, dtype=np.float32)
    mask = np.asarray(attention_mask)
    w_q = np.asarray(w_q, dtype=np.float32)
    w_k = np.asarray(w_k, dtype=np.float32)
    w_v = np.asarray(w_v, dtype=np.float32)
    w_o = np.asarray(w_o, dtype=np.float32)

    import ml_dtypes

    bf = ml_dtypes.bfloat16
    xt = x.transpose(0, 2, 1)  # [B, D, S] view
    # [B, NI, 128, NSC, 512] -> [B, NI, NSC, 128, 512]
    xtb = np.ascontiguousarray(
        xt.reshape(B, NI, 128, NSC, 512).transpose(0, 1, 3, 2, 4)
    ).astype(bf)

    wqT = w_q.T  # [i, d_out] view
    wkT = w_k.T
    wvT = w_v.T
    woT = w_o.T  # [hd, e] view

    m01 = mask.astype(np.float32)  # [B, S]
    mb = (NEGB * (1.0 - m01)).reshape(B, NST, 128).transpose(0, 2, 1)
    mb = np.ascontiguousarray(mb)  # [B, 128, NST]

    ki = np.arange(128)[:, None]
    wj = np.arange(128)[None, :]
    tri = (ki <= wj).astype(bf)  # [128, 128] 0/1 lower-triangle-inclusive

    # zmh[b, q] = 1 iff every causally-visible key of q is padded
    # (== the rows where the softmax denominator is exactly 0 on device)
    zmh_q = (np.cumsum(m01, axis=1) == 0.0).astype(bf)  # [B, S]
    zmh = np.broadcast_to(zmh_q[:, None, :], (B, 128, S))  # [B, 128, S]

    # mean of V rows over ALL keys, for the all-masked-row fallback
    xsum = x.sum(axis=1)  # [B, D]
    mv_full = (xsum @ w_v.T) / float(S)  # [B, D]

    ones = np.ones((128, 128), dtype=bf)

    in_maps = []
    xtb_slices = [
        np.ascontiguousarray(xtb[bg * BPC : (bg + 1) * BPC]) for bg in range(NBG)
    ]
    mb_slices = [
        np.ascontiguousarray(mb[bg * BPC : (bg + 1) * BPC]) for bg in range(NBG)
    ]
    zmh_slices = [
        np.ascontiguousarray(zmh[bg * BPC : (bg + 1) * BPC]) for bg in range(NBG)
    ]
    for c in range(8):
        hg, bg = c // 2, c % 2
        cols = slice(hg * HPC * DK, (hg + 1) * HPC * DK)
        # [2, NI, 128, 512]: per 2-head pass, wq cols then wk cols
        wqkt = np.stack(
            [
                np.concatenate(
                    [
                        wqT[:, (hg * HPC + 2 * p) * DK : (hg * HPC + 2 * p + 2) * DK],
                        wkT[:, (hg * HPC + 2 * p) * DK : (hg * HPC + 2 * p + 2) * DK],
                    ],
                    axis=1,
                ).reshape(NI, 128, 512)
                for p in range(2)
            ]
        ).astype(bf)
        wqkt = np.ascontiguousarray(wqkt)
        wvt = np.ascontiguousarray(
            wvT[:, cols].reshape(NI, 128, HPC * DK)
        ).astype(bf)
        # wott[ec, hd] = woT[this core's hd rows, ec-block] as [128, 512]
        wo_rows = woT[cols, :]  # [512, 2048]
        wott = np.ascontiguousarray(
            wo_rows.reshape(HPC, 128, NSC, 512).transpose(2, 0, 1, 3)
        ).astype(bf)
        mv = mv_full[bg * BPC : (bg + 1) * BPC, cols]  # [BPC, 512]
        mv = np.ascontiguousarray(
            mv.reshape(BPC, HPC, DK).transpose(0, 2, 1)
        )  # [BPC, 128, HPC]
        in_maps.append(
            {
                "xtb": xtb_slices[bg],
                "wqkt": wqkt,
                "wvt": wvt,
                "wott": wott,
                "mb": mb_slices[bg],
                "zmh": zmh_slices[bg],
                "tri": tri,
                "meanv": mv,
                "ones": ones,
            }
        )
    return in_maps


def kernel(x, attention_mask, w_q, w_k, w_v, w_o):
    _register_ntff_hook()
    from concourse.bass_utils import run_bass_kernel_spmd

    if "nc" not in _cache:
        _cache["nc"] = _build_program()
    nc = _cache["nc"]

    in_maps = _host_prep(x, attention_mask, w_q, w_k, w_v, w_o)

    trace = bool(int(os.environ.get("BASS_KERNEL_TRACE", "0")))
    res = run_bass_kernel_spmd(
        nc, in_maps, core_ids=list(range(8)), trace=trace
    )
    _cache["last_exec_time_ns"] = res.exec_time_ns
    _cache["last_results"] = res

    out = np.zeros((B, S, D), dtype=np.float32)
    for c in range(8):
        hg, bg = c // 2, c % 2
        part = np.asarray(res.results[c]["out"], dtype=np.float32)  # [BPC, S, D]
        out[bg * BPC : (bg + 1) * BPC] += part
    return out
